# revision 56
# baseline (speedup 1.0000x reference)
"""Two-layer GAT (PyG-style, eval mode) on 8 Trainium2 NeuronCores.

Strategy (dst-sharded, per-node edge columns):
  - Host: shard destination nodes into 8 contiguous ranges (one per core).
    Within a core, nodes are permuted by in-degree so that each window of 128
    nodes has near-uniform degree; window w stores its edges as K[w] columns
    of 128 rows, where row p holds the in-edges of the window's node p
    (padded to K[w] = max in-degree in the window, ~12% overhead).
    Self-loops are excluded from the edge lists (their features are local).
  - Device phase A: per-core rows of  rec1 = [x @ W1 + b1 | alpha_src | alpha_dst]
    via PE matmuls (alpha vectors folded into the weight matrix); the rec1
    AllGather is split into 4 quarter-table collectives, each fired as soon
    as its 5 windows of rows are computed, so the table exchange pipelines
    behind phase A.
  - Device phase B (layer-1 edge phase): windows are processed in GROUPS OF
    4; the 4 gathers of a group are issued back-to-back on SWDGE queues 0-3,
    whose descriptor generation runs on disjoint Q7 core pairs and therefore
    overlaps ~3x.  A small DVE copy stages each group's gather indices, which
    serializes the group's desc-gen burst after the previous group's vector
    work -- without this the gathers spread out uniformly and the DVE (which
    stalls hard while any SWDGE desc-gen is active: shared SBUF port) crawls
    through the whole phase.  Within a window: ex = exp(leaky_relu(a_s+a_d))
    without the segment-max shift (edge logits are bounded ~[-2,2], so this
    is numerically exact); pad edge columns gather a dedicated pad row whose
    alpha_src is -200 => exp ~ 4e-18, no masking.  alpha = ex/den is folded
    to bf16 BEFORE the big weighted multiply so the multiply runs at the
    DVE's 2x 16-bit rate; the segment sum is one strided tensor_reduce.
    rec2 = [h @ W2 | alpha2_src | alpha2_dst] rows stream out per window and
    the rec2 AllGather fires in quarters as their windows complete.
  - Phase D repeats the grouped edge phase for layer 2 (1 head, 40 ch),
    buffering per-window logits on chip; log_softmax runs as one batched
    tail pass.  The host inverse-permutes the rows.

    Layer-1 records are 768 B: 256 bf16 features + 16 fp32 alphas + pad
    (gather elements must be 256B-multiples).  Measured bottlenecks: SWDGE
    descriptor emission (~8.7 ns per gathered row serial, ~2.8x when 4
    queues emit concurrently) and the DVE stall during desc-gen.
"""

import sys

for _p in ("/opt/trn_rl_repo", "/root/.axon_site/_ro/trn_rl_repo"):
    if _p not in sys.path:
        sys.path.append(_p)

import numpy as np

import concourse.bass as bass
import concourse.mybir as mybir
import concourse.tile as tile
from concourse import bacc
from concourse.bass_utils import run_bass_kernel_spmd
from concourse.masks import make_identity

F32 = mybir.dt.float32
I16 = mybir.dt.int16
BF16 = mybir.dt.bfloat16
FP8 = mybir.dt.float8e4
ALU = mybir.AluOpType
ACTF = mybir.ActivationFunctionType
AXX = mybir.AxisListType.X

CORES = 8
PW = 128           # nodes per window
NEG_SLOPE = 0.2
GRP = 4            # windows per gather group == SWDGE queues used

_CACHE = {}


# --------------------------------------------------------------------------
# host-side preprocessing
# --------------------------------------------------------------------------

def _wrap_idx(vals):
    """Wrap a flat index vector into the [128, n/16] layout dma_gather wants
    (index i at partition i%16, col i//16; replicated to all 8 Q7 groups)."""
    w = vals.reshape(-1, 16).T.astype(np.int16)
    return np.ascontiguousarray(np.tile(w, (8, 1)))


def _prep(x, edge_index, W1, a1_src, a1_dst, b1, W2, a2_src, a2_dst, b2):
    N, F = x.shape
    H, C = a1_src.shape
    OUT = W2.shape[1]
    NPC = N // CORES
    W = (NPC + PW - 1) // PW
    NPCP = W * PW
    if NPCP == NPC:
        NPCP += PW          # spare pad block (holds the -200 pad row)
        W += 1

    # quarter-split of the node tables: NQ contiguous row blocks per core,
    # stored [quarter, core, QH] so each AllGather's output is contiguous.
    NQ = 4 if W % 4 == 0 else (2 if W % 2 == 0 else 1)
    WQ = W // NQ
    QH = WQ * PW
    assert NPC > (NQ - 1) * QH   # pad rows live in the last quarter

    src0 = np.asarray(edge_index[0], np.int64)
    dst0 = np.asarray(edge_index[1], np.int64)

    deg = np.zeros(N, np.int64)
    np.add.at(deg, dst0, 1)

    # per-core degree-sorted node permutation; perm[new_global_row] = node id
    perm = np.empty(N, np.int64)
    for c in range(CORES):
        ids = np.arange(c * NPC, (c + 1) * NPC)
        perm[c * NPC:(c + 1) * NPC] = ids[np.argsort(deg[ids], kind="stable")]
    newrow = np.empty(N, np.int64)
    newrow[perm] = np.arange(N)          # node id -> permuted global row
    # tab1 rows: [core, NPCP] (single AllGather); tab2 rows: [quarter, core,
    # QH] so its AllGather can fire in quarters as windows complete.
    _c = newrow // NPC
    _r = newrow % NPC
    tabrow = _c * NPCP + _r
    tabrow2 = (_r // QH) * (CORES * QH) + _c * QH + (_r % QH)

    # per-window max degree (shared across cores for SPMD uniformity)
    degw = deg[perm].reshape(CORES, NPC)
    K = []
    for w in range(W):
        lo, hi = w * PW, min((w + 1) * PW, NPC)
        if lo >= hi:
            K.append(1)
        else:
            K.append(max(1, int(degw[:, lo:hi].max())))
    K = np.asarray(K, np.int64)
    CK = np.concatenate([[0], np.cumsum(K)])   # column offsets
    SK = int(K.sum())
    EPC = SK * PW                              # gather indices per core/layer

    # in-edges grouped by (permuted) destination row: CSR over new rows
    order = np.argsort(newrow[dst0], kind="stable")
    src_s = src0[order]
    starts = np.zeros(N + 1, np.int64)
    np.add.at(starts, newrow[dst0] + 1, 1)
    starts = np.cumsum(starts)

    pq, po = NPC // QH, NPC % QH               # pad row quarter/offset
    isrc = np.zeros((CORES, EPC), np.int64)
    isrc2 = np.zeros((CORES, EPC), np.int64)
    for c in range(CORES):
        # pads gather the core's first pad row (alpha_src overwritten to -200)
        isrc[c, :] = c * NPCP + NPC
        isrc2[c, :] = pq * (CORES * QH) + c * QH + po
        for w in range(W):
            kw = int(K[w])
            for p in range(PW):
                lp = w * PW + p
                if lp >= NPC:
                    continue
                r = c * NPC + lp
                s, e = starts[r], starts[r + 1]
                d = e - s
                cols = CK[w] + np.arange(d)
                isrc[c, cols * PW + p] = tabrow[src_s[s:e]]
                isrc2[c, cols * PW + p] = tabrow2[src_s[s:e]]

    isrc_w = np.stack([_wrap_idx(isrc[c]) for c in range(CORES)])
    isrc2_w = np.stack([_wrap_idx(isrc2[c]) for c in range(CORES)])

    # folded weight matrices (f64 for exactness of the tiny folds)
    As = np.zeros((F, H), np.float64)
    Ad = np.zeros((F, H), np.float64)
    for h in range(H):
        As[h * C:(h + 1) * C, h] = a1_src[h]
        Ad[h * C:(h + 1) * C, h] = a1_dst[h]
    W1_64 = np.asarray(W1, np.float64)
    wc1 = np.concatenate(
        [np.asarray(W1, np.float32),
         (W1_64 @ As).astype(np.float32),
         (W1_64 @ Ad).astype(np.float32)], 1)           # [F, F+2H]
    W2_64 = np.asarray(W2, np.float64)
    wc2 = np.concatenate(
        [np.asarray(W2, np.float32),
         (W2_64 @ np.asarray(a2_src[0], np.float64))[:, None].astype(np.float32),
         (W2_64 @ np.asarray(a2_dst[0], np.float64))[:, None].astype(np.float32)],
        1)                                               # [F, OUT+2]

    KT = F // 128
    xt = np.zeros((CORES, KT, 128, NPCP), np.float32)
    xp = np.asarray(x, np.float32)[perm]
    for c in range(CORES):
        xt[c, :, :, :NPC] = (xp[c * NPC:(c + 1) * NPC].T
                             .reshape(KT, 128, NPC))
    # bf16 inputs for the phase-A matmuls (PE accumulates in f32)
    import ml_dtypes
    xt_bf = xt.astype(ml_dtypes.bfloat16)
    wc1_bf = wc1.astype(ml_dtypes.bfloat16)

    host = {
        "N": N, "F": F, "H": H, "C": C, "OUT": OUT,
        "NPC": NPC, "W": W, "NPCP": NPCP, "NQ": NQ, "QH": QH, "WQ": WQ,
        "K": tuple(int(k) for k in K), "SK": SK, "EPC": EPC,
        "perm": perm,
        "use_b1": bool(np.any(b1)), "use_b2": bool(np.any(b2)),
    }
    in_maps = []
    for c in range(CORES):
        m = {
            "xt": xt_bf[c],
            "wc1": np.ascontiguousarray(wc1_bf.reshape(2, F // 2, F + 2 * H)),
            "wc2": np.ascontiguousarray(wc2.reshape(2, F // 2, OUT + 2)),
            "isrc": isrc_w[c],
            "isrc2": isrc2_w[c],
        }
        if host["use_b1"]:
            m["b1rep"] = np.broadcast_to(np.asarray(b1, np.float32), (PW, F)).copy()
        if host["use_b2"]:
            m["b2rep"] = np.broadcast_to(np.asarray(b2, np.float32), (PW, OUT)).copy()
        in_maps.append(m)
    return host, in_maps


# --------------------------------------------------------------------------
# device kernel
# --------------------------------------------------------------------------

def _build(hp):
    F, H, C, OUT = hp["F"], hp["H"], hp["C"], hp["OUT"]
    W, NPCP = hp["W"], hp["NPCP"]
    NQ, QH, WQ = hp["NQ"], hp["QH"], hp["WQ"]
    K, SK, EPC = hp["K"], hp["SK"], hp["EPC"]
    KT = F // 128               # K tiles (2)
    # layer-1 record, 512 bytes: [256 x fp8 features | 16 x f32 alphas | pad]
    REC1 = 512                  # fp8 (byte) units
    assert REC1 % 256 == 0 and F + 64 <= REC1
    A1 = F + 64                 # valid bytes (features + alpha area)
    AF = F + 2 * H              # 272 f32 matmul output cols
    REC2 = 128                  # bf16 units = 256 B rec2 rows
    A2 = OUT + 4                # valid bf16 cols: 40 h2 + 2 f32 alphas
    A2M = OUT + 2               # f32 matmul output cols of wc2
    NTAB = CORES * NPCP
    EPC16 = EPC // 16
    NPC = hp["NPC"]
    CK = [0]
    for k in K:
        CK.append(CK[-1] + k)

    # window processing order (largest degree first) and groups of GRP
    order = list(reversed(range(W)))
    groups = [order[i:i + GRP] for i in range(0, W, GRP)]

    nc = bacc.Bacc(None, target_bir_lowering=False, num_swdge_queues=GRP)

    xt_p = nc.declare_dram_parameter("xt", [KT, 128, NPCP], BF16, isOutput=False)
    wc1_p = nc.declare_dram_parameter("wc1", [KT, 128, AF], BF16, isOutput=False)
    wc2_p = nc.declare_dram_parameter("wc2", [KT, 128, A2M], F32, isOutput=False)
    isrc_p = nc.declare_dram_parameter("isrc", [128, EPC16], I16, isOutput=False)
    isrc2_p = nc.declare_dram_parameter("isrc2", [128, EPC16], I16, isOutput=False)
    b1_p = (nc.declare_dram_parameter("b1rep", [PW, F], F32, isOutput=False)
            if hp["use_b1"] else None)
    b2_p = (nc.declare_dram_parameter("b2rep", [PW, OUT], F32, isOutput=False)
            if hp["use_b2"] else None)
    out_p = nc.declare_dram_parameter("out", [NPCP, OUT], F32, isOutput=True)

    with tile.TileContext(nc) as tc:
        with (
            tc.tile_pool(name="dram", bufs=1, space="DRAM") as dram,
            tc.tile_pool(name="const", bufs=1) as cpool,
            tc.tile_pool(name="gath", bufs=3) as gp,
            tc.tile_pool(name="mid", bufs=2) as mp,
            tc.tile_pool(name="psA", bufs=4, space="PSUM") as psA,
            tc.tile_pool(name="psB", bufs=2, space="PSUM") as psB,
        ):
            r1loc = dram.tile([NPCP, REC1], FP8)
            tab1 = dram.tile([NTAB, REC1], FP8, addr_space="Shared")
            r2loc = dram.tile([NPCP, REC2], BF16)
            tab2 = dram.tile([NTAB, REC2], BF16, addr_space="Local")

            # resident constants
            wc1_sb = cpool.tile([128, KT, AF], BF16)
            for g in range(KT):
                nc.sync.dma_start(out=wc1_sb[:, g, :], in_=wc1_p[g])
            xt_sb = cpool.tile([128, KT, NPCP], BF16)
            for g in range(KT):
                nc.sync.dma_start(out=xt_sb[:, g, :], in_=xt_p[g])
            wc2_sb = cpool.tile([128, KT, A2M], F32)
            for g in range(KT):
                nc.sync.dma_start(out=wc2_sb[:, g, :], in_=wc2_p[g])
            ident = cpool.tile([PW, PW], F32)
            make_identity(nc, ident[:])
            isrc_sb = cpool.tile([128, EPC16], I16)
            nc.sync.dma_start(out=isrc_sb[:], in_=isrc_p[:])
            isrc2_sb = cpool.tile([128, EPC16], I16)
            nc.sync.dma_start(out=isrc2_sb[:], in_=isrc2_p[:])
            if b1_p is not None:
                b1_sb = cpool.tile([PW, F], F32)
                nc.sync.dma_start(out=b1_sb[:], in_=b1_p[:])
            if b2_p is not None:
                b2_sb = cpool.tile([PW, OUT], F32)
                nc.sync.dma_start(out=b2_sb[:], in_=b2_p[:])
            pcA = cpool.tile([PW, 16], F32)      # pad-row alphas for rec1
            nc.vector.memset(pcA[:], 0.0)
            nc.vector.memset(pcA[:, :H], -200.0)
            pzA = cpool.tile([PW, F], FP8)       # pad-row features (zero)
            nc.vector.memset(pzA[:], 0.0)
            pcB = cpool.tile([PW, 2], F32)       # pad-row alphas for rec2
            nc.vector.memset(pcB[:], 0.0)
            nc.vector.memset(pcB[:, :1], -200.0)
            pzB = cpool.tile([PW, OUT], BF16)
            nc.vector.memset(pzB[:], 0.0)
            lgall = cpool.tile([128, W, OUT], F32)
            ezall = cpool.tile([128, W, OUT], F32)
            ssall = cpool.tile([128, W], F32)
            lsall = cpool.tile([128, W], F32)

            # ---------------- phase A: rec1 rows for this core ------------
            # (gathered pad columns [A1:REC1) / [A2:REC2) are never read, so
            # the tables are left uninitialized there)
            for nt in range(W):
                rp = psB.tile([128, AF], F32, tag="acc")
                for g in range(KT):
                    nc.tensor.matmul(rp[:], lhsT=xt_sb[:, g, nt * PW:(nt + 1) * PW],
                                     rhs=wc1_sb[:, g, :],
                                     start=(g == 0), stop=(g == KT - 1))
                rs = mp.tile([128, A1], FP8, tag="rs")
                nc.vector.tensor_copy(out=rs[:, F:F + 64].bitcast(F32),
                                      in_=rp[:, F:])
                if b1_p is not None:
                    # bias folded into the aggregated features (sum alpha = 1)
                    nc.vector.tensor_add(out=rs[:, :F], in0=rp[:, :F], in1=b1_sb[:])
                else:
                    nc.vector.tensor_copy(out=rs[:, :F], in_=rp[:, :F])
                nc.sync.dma_start(out=r1loc[nt * PW:(nt + 1) * PW, :A1], in_=rs[:])
                if nt == W - 1:
                    # pad rows: zero features, alpha_src=-200 (exp ~ 4e-18)
                    for lo in range(NPC, NPCP, PW):
                        nr = min(PW, NPCP - lo)
                        nc.sync.dma_start(out=r1loc[lo:lo + nr, :F],
                                          in_=pzA[:nr])
                        nc.sync.dma_start(
                            out=r1loc[lo:lo + nr, F:F + 64].bitcast(F32),
                            in_=pcA[:nr])
                    # single AllGather: per-op fixed cost (~25us) makes
                    # quarter-split collectives a net loss for this table.
                    nc.gpsimd.collective_compute(
                        "AllGather", ALU.bypass,
                        replica_groups=[list(range(CORES))],
                        ins=[r1loc[:, :].opt()],
                        outs=[tab1[:, :].opt()])

            # ---------------- phase B: layer-1 edge phase -----------------
            cc2_fired = [False] * NQ

            def fire_cc2(done_min):
                for q in range(NQ - 1, -1, -1):
                    if cc2_fired[q] or done_min > q * WQ:
                        continue
                    cc2_fired[q] = True
                    nc.gpsimd.collective_compute(
                        "AllGather", ALU.bypass,
                        replica_groups=[list(range(CORES))],
                        ins=[r2loc[q * QH:(q + 1) * QH, :].opt()],
                        outs=[tab2[q * CORES * QH:(q + 1) * CORES * QH, :].opt()])

            # self-loop attention terms for all windows in one shot:
            # exp(leaky_relu(a_s + a_d)) on [128, W, H]
            aall = cpool.tile([128, W, 16], F32)
            nc.sync.dma_start(
                out=aall[:],
                in_=r1loc[:W * PW, F:F + 64].bitcast(F32)
                    .rearrange("(w p) a -> p w a", p=PW))
            essall = cpool.tile([128, W, H], F32)
            nc.vector.tensor_tensor(out=essall[:], in0=aall[:, :, :H],
                                    in1=aall[:, :, H:], op=ALU.add)
            nc.vector.scalar_tensor_tensor(
                out=essall[:], in0=essall[:], scalar=NEG_SLOPE, in1=essall[:],
                op0=ALU.mult, op1=ALU.max)
            nc.scalar.activation(out=essall[:], in_=essall[:], func=ACTF.Exp)

            for w in order:
                kw = K[w]
                # the window's gather is split over the 4 SWDGE queues
                # (disjoint column ranges of one tile): descriptor generation
                # runs on all 4 Q7 core pairs concurrently (~3x), and the
                # per-window ring population keeps transfers window-serial so
                # consumers pipeline behind them.
                G1 = gp.tile([128, kw, REC1], FP8, tag="G1")
                qof = [kw * q // GRP for q in range(GRP + 1)]
                for q in range(GRP):
                    kq = qof[q + 1] - qof[q]
                    if kq == 0:
                        continue
                    nc.gpsimd.dma_gather(
                        G1[:, qof[q]:qof[q + 1], :], tab1[:, :],
                        isrc_sb[:, (CK[w] + qof[q]) * 8:(CK[w] + qof[q + 1]) * 8],
                        kq * PW, kq * PW, REC1, single_packet=False,
                        queue_num=q)
                if True:
                    loc = mp.tile([128, F], FP8, tag="loc")
                    nc.sync.dma_start(out=loc[:],
                                      in_=r1loc[w * PW:(w + 1) * PW, :F])
                    # ex = exp(leaky_relu(a_s[src] + a_d[dst]))  (pads -> ~0)
                    es_t = mp.tile([128, kw, H], F32, tag="es")
                    es = es_t[:, :, :]
                    nc.vector.tensor_tensor(
                        out=es, in0=G1[:, :, F:F + 32].bitcast(F32),
                        in1=aall[:, w, H:].unsqueeze(1)
                            .to_broadcast([128, kw, H]),
                        op=ALU.add)
                    nc.vector.scalar_tensor_tensor(
                        out=es, in0=es, scalar=NEG_SLOPE, in1=es,
                        op0=ALU.mult, op1=ALU.max)
                    nc.scalar.activation(out=es, in_=es, func=ACTF.Exp)
                    # alpha = ex / (sum_t ex + ex_self), folded to bf16 so the
                    # big multiply runs at the DVE 16-bit rate
                    den = mp.tile([128, H], F32, tag="den")
                    nc.vector.tensor_reduce(
                        out=den[:], in_=es.rearrange("p t h -> p h t"),
                        axis=AXX, op=ALU.add)
                    nc.vector.tensor_add(out=den[:], in0=den[:],
                                         in1=essall[:, w, :])
                    rcp = mp.tile([128, H], F32, tag="rcp")
                    nc.vector.reciprocal(rcp[:], den[:])
                    abf = mp.tile([128, kw, H], BF16, tag="abf")
                    nc.vector.tensor_tensor(
                        out=abf[:], in0=es,
                        in1=rcp[:].unsqueeze(1).to_broadcast([128, kw, H]),
                        op=ALU.mult)
                    asf = mp.tile([128, H], F32, tag="asf")
                    nc.vector.tensor_tensor(out=asf[:], in0=essall[:, w, :],
                                            in1=rcp[:], op=ALU.mult)
                    # weighted feature sum over edge columns + self
                    GW = mp.tile([128, kw, F], BF16, tag="GW")
                    nc.vector.tensor_tensor(
                        out=GW[:].rearrange("p t (h c) -> p t h c", h=H),
                        in0=G1[:, :, :F].rearrange("p t (h c) -> p t h c", h=H),
                        in1=abf[:].unsqueeze(3).to_broadcast([128, kw, H, C]),
                        op=ALU.mult)
                    # pairwise-tree segment sum over t: contiguous bf16 adds
                    # run at the DVE 16-bit rate (the strided f32 tensor_reduce
                    # over the t axis is ~2.5x slower)
                    n = kw
                    while n > 1:
                        if n % 2 == 1:
                            nc.vector.tensor_tensor(
                                out=GW[:, 0, :], in0=GW[:, 0, :],
                                in1=GW[:, n - 1, :], op=ALU.add)
                        h2 = n // 2
                        nc.vector.tensor_tensor(
                            out=GW[:, :h2, :], in0=GW[:, :h2, :],
                            in1=GW[:, h2:2 * h2, :], op=ALU.add)
                        n = h2
                    tmp = mp.tile([128, F], F32, tag="tmp")
                    nc.vector.tensor_tensor(
                        out=tmp[:].rearrange("p (h c) -> p h c", h=H),
                        in0=loc[:].rearrange("p (h c) -> p h c", h=H),
                        in1=asf[:].unsqueeze(2).to_broadcast([128, H, C]),
                        op=ALU.mult)
                    ho = mp.tile([128, F], F32, tag="ho")
                    nc.vector.tensor_add(out=ho[:], in0=GW[:, 0, :], in1=tmp[:])
                    # ELU(x) = relu(x) + exp(-relu(-x)) - 1, with all three
                    # activations on the (otherwise idle) scalar engine
                    xm = mp.tile([128, F], F32, tag="xm")
                    nc.scalar.activation(out=xm[:], in_=ho[:], func=ACTF.Relu,
                                         scale=-1.0)
                    nc.scalar.activation(out=xm[:], in_=xm[:], func=ACTF.Exp,
                                         scale=-1.0)
                    nc.scalar.activation(out=ho[:], in_=ho[:], func=ACTF.Relu)
                    nc.vector.scalar_tensor_tensor(
                        out=ho[:], in0=ho[:], scalar=-1.0, in1=xm[:],
                        op0=ALU.add, op1=ALU.add)
                    hT = mp.tile([128, KT, 128], F32, tag="hT")
                    for g in range(KT):
                        tp = psA.tile([128, 128], F32, tag="tp")
                        nc.tensor.transpose(out=tp[:],
                                            in_=ho[:, g * 128:(g + 1) * 128],
                                            identity=ident[:])
                        nc.vector.tensor_copy(out=hT[:, g, :], in_=tp[:])
                    r2p = psB.tile([128, A2M], F32, tag="acc2")
                    for g in range(KT):
                        nc.tensor.matmul(r2p[:], lhsT=hT[:, g, :],
                                         rhs=wc2_sb[:, g, :],
                                         start=(g == 0), stop=(g == KT - 1))
                    r2sb = mp.tile([128, A2], BF16, tag="r2sb")
                    nc.vector.tensor_copy(out=r2sb[:, :OUT], in_=r2p[:, :OUT])
                    nc.vector.tensor_copy(
                        out=r2sb[:, OUT:OUT + 4].bitcast(F32),
                        in_=r2p[:, OUT:OUT + 2])
                    nc.sync.dma_start(out=r2loc[w * PW:(w + 1) * PW, :A2],
                                      in_=r2sb[:])
                    if w == W - 1 and NPCP > NPC:
                        # pad rows (inside the last quarter's row range)
                        for lo in range(NPC, NPCP, PW):
                            nr = min(PW, NPCP - lo)
                            nc.sync.dma_start(out=r2loc[lo:lo + nr, :OUT],
                                              in_=pzB[:nr])
                            nc.sync.dma_start(
                                out=r2loc[lo:lo + nr, OUT:OUT + 4].bitcast(F32),
                                in_=pcB[:nr])
                fire_cc2(w)

            # ---------------- phase D: layer-2 edge phase -----------------
            # batched self-loop attention terms from the rec2 alpha columns
            a2all = cpool.tile([128, W, 2], F32)
            nc.sync.dma_start(
                out=a2all[:],
                in_=r2loc[:W * PW, OUT:OUT + 4].bitcast(F32)
                    .rearrange("(w p) a -> p w a", p=PW))
            ess2all = cpool.tile([128, W], F32)
            nc.vector.tensor_tensor(out=ess2all[:], in0=a2all[:, :, 0],
                                    in1=a2all[:, :, 1], op=ALU.add)
            nc.vector.scalar_tensor_tensor(
                out=ess2all[:], in0=ess2all[:], scalar=NEG_SLOPE,
                in1=ess2all[:], op0=ALU.mult, op1=ALU.max)
            nc.scalar.activation(out=ess2all[:], in_=ess2all[:], func=ACTF.Exp)

            for w in order:
                kw = K[w]
                G2 = gp.tile([128, kw, REC2], BF16, tag="G2")
                qof = [kw * q // GRP for q in range(GRP + 1)]
                for q in range(GRP):
                    kq = qof[q + 1] - qof[q]
                    if kq == 0:
                        continue
                    nc.gpsimd.dma_gather(
                        G2[:, qof[q]:qof[q + 1], :], tab2[:, :],
                        isrc2_sb[:, (CK[w] + qof[q]) * 8:(CK[w] + qof[q + 1]) * 8],
                        kq * PW, kq * PW, REC2, single_packet=False,
                        queue_num=q)
                if True:
                    loc2 = mp.tile([128, OUT], BF16, tag="loc2")
                    nc.sync.dma_start(out=loc2[:],
                                      in_=r2loc[w * PW:(w + 1) * PW, :OUT])
                    es2_t = mp.tile([128, kw, 1], F32, tag="es2")
                    es2 = es2_t[:, :, :]
                    nc.vector.tensor_tensor(
                        out=es2, in0=G2[:, :, OUT:OUT + 2].bitcast(F32),
                        in1=a2all[:, w, 1:2].unsqueeze(1)
                            .to_broadcast([128, kw, 1]),
                        op=ALU.add)
                    nc.vector.scalar_tensor_tensor(
                        out=es2, in0=es2, scalar=NEG_SLOPE, in1=es2,
                        op0=ALU.mult, op1=ALU.max)
                    nc.scalar.activation(out=es2, in_=es2, func=ACTF.Exp)
                    den2 = mp.tile([128, 1], F32, tag="den2")
                    nc.vector.tensor_reduce(
                        out=den2[:], in_=es2.rearrange("p t h -> p h t"),
                        axis=AXX, op=ALU.add)
                    nc.vector.tensor_add(out=den2[:], in0=den2[:],
                                         in1=ess2all[:, w:w + 1])
                    rcp2 = mp.tile([128, 1], F32, tag="rcp2")
                    nc.vector.reciprocal(rcp2[:], den2[:])
                    nc.vector.tensor_scalar_mul(out=es2, in0=es2,
                                                scalar1=rcp2[:, :1])
                    nc.vector.tensor_tensor(
                        out=G2[:, :, :OUT], in0=G2[:, :, :OUT],
                        in1=es2.to_broadcast([128, kw, OUT]), op=ALU.mult)
                    num2 = mp.tile([128, OUT], F32, tag="num2")
                    nc.vector.tensor_reduce(
                        out=num2[:], in_=G2[:, :, :OUT].rearrange("p t f -> p f t"),
                        axis=AXX, op=ALU.add)
                    tmp2 = mp.tile([128, OUT], F32, tag="tmp2")
                    nc.vector.tensor_scalar(
                        out=tmp2[:], in0=loc2[:], scalar1=ess2all[:, w:w + 1],
                        scalar2=rcp2[:, :1], op0=ALU.mult, op1=ALU.mult)
                    nc.vector.tensor_add(out=lgall[:, w, :], in0=num2[:],
                                         in1=tmp2[:])
                    if b2_p is not None:
                        nc.vector.tensor_add(out=lgall[:, w, :],
                                             in0=lgall[:, w, :], in1=b2_sb[:])

            # batched log_softmax over all windows (no max-shift: logits are
            # bounded ~[-2, 2]); one Exp over [128, W, OUT], one reduce, one Ln.
            nc.scalar.activation(out=ezall[:], in_=lgall[:], func=ACTF.Exp)
            nc.vector.tensor_reduce(out=ssall[:], in_=ezall[:],
                                    axis=AXX, op=ALU.add)
            nc.scalar.activation(out=lsall[:], in_=ssall[:], func=ACTF.Ln)
            nc.vector.tensor_tensor(
                out=lgall[:], in0=lgall[:],
                in1=lsall[:].unsqueeze(2).to_broadcast([128, W, OUT]),
                op=ALU.subtract)
            nc.sync.dma_start(
                out=out_p[:W * PW].rearrange("(w p) o -> p w o", p=PW),
                in_=lgall[:])

    nc.compile()
    return nc


# --------------------------------------------------------------------------
# public entry point
# --------------------------------------------------------------------------

def kernel(x, edge_index, W1, a1_src, a1_dst, b1, W2, a2_src, a2_dst, b2,
           _want_trace=False):
    x = np.asarray(x)
    host, in_maps = _prep(x, np.asarray(edge_index), np.asarray(W1),
                          np.asarray(a1_src), np.asarray(a1_dst),
                          np.asarray(b1), np.asarray(W2), np.asarray(a2_src),
                          np.asarray(a2_dst), np.asarray(b2))
    key = (host["N"], host["F"], host["H"], host["C"], host["OUT"],
           host["K"], host["NQ"], host["use_b1"], host["use_b2"])
    if key not in _CACHE:
        _CACHE[key] = _build(host)
    nc = _CACHE[key]
    res = run_bass_kernel_spmd(nc, in_maps, core_ids=list(range(CORES)),
                               trace=_want_trace)
    NPC = host["NPC"]
    out = np.empty((host["N"], host["OUT"]), np.float32)
    for c in range(CORES):
        out[host["perm"][c * NPC:(c + 1) * NPC]] = res.results[c]["out"][:NPC]
    if _want_trace:
        kernel._last_result = res
    return np.ascontiguousarray(out)


# revision 57
# speedup vs baseline: 1.1118x; 1.1118x over previous
"""Two-layer GAT (PyG-style, eval mode) on 8 Trainium2 NeuronCores.

Strategy (dst-sharded, per-node edge columns):
  - Host: shard destination nodes into 8 contiguous ranges (one per core).
    Within a core, nodes are permuted by in-degree so that each window of 128
    nodes has near-uniform degree; window w stores its edges as K[w] columns
    of 128 rows, where row p holds the in-edges of the window's node p
    (padded to K[w] = max in-degree in the window, ~12% overhead).
    Self-loops are excluded from the edge lists (their features are local).
  - Device phase A: per-core rows of  rec1 = [x @ W1 + b1 | alpha_src | alpha_dst]
    via PE matmuls (alpha vectors folded into the weight matrix); the rec1
    AllGather is split into 4 quarter-table collectives, each fired as soon
    as its 5 windows of rows are computed, so the table exchange pipelines
    behind phase A.
  - Device phase B (layer-1 edge phase): windows are processed in GROUPS OF
    4; the 4 gathers of a group are issued back-to-back on SWDGE queues 0-3,
    whose descriptor generation runs on disjoint Q7 core pairs and therefore
    overlaps ~3x.  A small DVE copy stages each group's gather indices, which
    serializes the group's desc-gen burst after the previous group's vector
    work -- without this the gathers spread out uniformly and the DVE (which
    stalls hard while any SWDGE desc-gen is active: shared SBUF port) crawls
    through the whole phase.  Within a window: ex = exp(leaky_relu(a_s+a_d))
    without the segment-max shift (edge logits are bounded ~[-2,2], so this
    is numerically exact); pad edge columns gather a dedicated pad row whose
    alpha_src is -200 => exp ~ 4e-18, no masking.  alpha = ex/den is folded
    to bf16 BEFORE the big weighted multiply so the multiply runs at the
    DVE's 2x 16-bit rate; the segment sum is one strided tensor_reduce.
    rec2 = [h @ W2 | alpha2_src | alpha2_dst] rows stream out per window and
    the rec2 AllGather fires in quarters as their windows complete.
  - Phase D repeats the grouped edge phase for layer 2 (1 head, 40 ch),
    buffering per-window logits on chip; log_softmax runs as one batched
    tail pass.  The host inverse-permutes the rows.

    Layer-1 records are 768 B: 256 bf16 features + 16 fp32 alphas + pad
    (gather elements must be 256B-multiples).  Measured bottlenecks: SWDGE
    descriptor emission (~8.7 ns per gathered row serial, ~2.8x when 4
    queues emit concurrently) and the DVE stall during desc-gen.
"""

import sys

for _p in ("/opt/trn_rl_repo", "/root/.axon_site/_ro/trn_rl_repo"):
    if _p not in sys.path:
        sys.path.append(_p)

import numpy as np

import concourse.bass as bass
import concourse.mybir as mybir
import concourse.tile as tile
from concourse import bacc
from concourse.bass_utils import run_bass_kernel_spmd
from concourse.masks import make_identity

F32 = mybir.dt.float32
I16 = mybir.dt.int16
BF16 = mybir.dt.bfloat16
FP8 = mybir.dt.float8e4
ALU = mybir.AluOpType
ACTF = mybir.ActivationFunctionType
AXX = mybir.AxisListType.X

CORES = 8
PW = 128           # nodes per window
NEG_SLOPE = 0.2
GRP = 4            # windows per gather group == SWDGE queues used

_CACHE = {}


# --------------------------------------------------------------------------
# host-side preprocessing
# --------------------------------------------------------------------------

def _wrap_idx(vals):
    """Wrap a flat index vector into the [128, n/16] layout dma_gather wants
    (index i at partition i%16, col i//16; replicated to all 8 Q7 groups)."""
    w = vals.reshape(-1, 16).T.astype(np.int16)
    return np.ascontiguousarray(np.tile(w, (8, 1)))


def _prep(x, edge_index, W1, a1_src, a1_dst, b1, W2, a2_src, a2_dst, b2):
    N, F = x.shape
    H, C = a1_src.shape
    OUT = W2.shape[1]
    NPC = N // CORES
    W = (NPC + PW - 1) // PW
    NPCP = W * PW
    if NPCP == NPC:
        NPCP += PW          # spare pad block (holds the -200 pad row)
        W += 1

    # quarter-split of the node tables: NQ contiguous row blocks per core,
    # stored [quarter, core, QH] so each AllGather's output is contiguous.
    NQ = 4 if W % 4 == 0 else (2 if W % 2 == 0 else 1)
    WQ = W // NQ
    QH = WQ * PW
    assert NPC > (NQ - 1) * QH   # pad rows live in the last quarter

    src0 = np.asarray(edge_index[0], np.int64)
    dst0 = np.asarray(edge_index[1], np.int64)

    deg = np.zeros(N, np.int64)
    np.add.at(deg, dst0, 1)

    # per-core degree-sorted node permutation; perm[new_global_row] = node id
    perm = np.empty(N, np.int64)
    for c in range(CORES):
        ids = np.arange(c * NPC, (c + 1) * NPC)
        perm[c * NPC:(c + 1) * NPC] = ids[np.argsort(deg[ids], kind="stable")]
    newrow = np.empty(N, np.int64)
    newrow[perm] = np.arange(N)          # node id -> permuted global row
    # tab1 rows: [core, NPCP] (single AllGather); tab2 rows: [quarter, core,
    # QH] so its AllGather can fire in quarters as windows complete.
    _c = newrow // NPC
    _r = newrow % NPC
    tabrow = _c * NPCP + _r
    tabrow2 = (_r // QH) * (CORES * QH) + _c * QH + (_r % QH)

    # per-window max degree (shared across cores for SPMD uniformity)
    degw = deg[perm].reshape(CORES, NPC)
    K = []
    for w in range(W):
        lo, hi = w * PW, min((w + 1) * PW, NPC)
        if lo >= hi:
            K.append(1)
        else:
            K.append(max(1, int(degw[:, lo:hi].max())))
    K = np.asarray(K, np.int64)
    CK = np.concatenate([[0], np.cumsum(K)])   # column offsets
    SK = int(K.sum())
    EPC = SK * PW                              # gather indices per core/layer

    # in-edges grouped by (permuted) destination row: CSR over new rows
    order = np.argsort(newrow[dst0], kind="stable")
    src_s = src0[order]
    starts = np.zeros(N + 1, np.int64)
    np.add.at(starts, newrow[dst0] + 1, 1)
    starts = np.cumsum(starts)

    pq, po = NPC // QH, NPC % QH               # pad row quarter/offset
    isrc = np.zeros((CORES, EPC), np.int64)
    isrc2 = np.zeros((CORES, EPC), np.int64)
    for c in range(CORES):
        # pads gather the core's first pad row (alpha_src overwritten to -200)
        isrc[c, :] = c * NPCP + NPC
        isrc2[c, :] = pq * (CORES * QH) + c * QH + po
        for w in range(W):
            kw = int(K[w])
            for p in range(PW):
                lp = w * PW + p
                if lp >= NPC:
                    continue
                r = c * NPC + lp
                s, e = starts[r], starts[r + 1]
                d = e - s
                cols = CK[w] + np.arange(d)
                isrc[c, cols * PW + p] = tabrow[src_s[s:e]]
                isrc2[c, cols * PW + p] = tabrow2[src_s[s:e]]

    isrc_w = np.stack([_wrap_idx(isrc[c]) for c in range(CORES)])
    isrc2_w = np.stack([_wrap_idx(isrc2[c]) for c in range(CORES)])

    # folded weight matrices (f64 for exactness of the tiny folds)
    As = np.zeros((F, H), np.float64)
    Ad = np.zeros((F, H), np.float64)
    for h in range(H):
        As[h * C:(h + 1) * C, h] = a1_src[h]
        Ad[h * C:(h + 1) * C, h] = a1_dst[h]
    W1_64 = np.asarray(W1, np.float64)
    wc1 = np.concatenate(
        [np.asarray(W1, np.float32),
         (W1_64 @ As).astype(np.float32),
         (W1_64 @ Ad).astype(np.float32)], 1)           # [F, F+2H]
    W2_64 = np.asarray(W2, np.float64)
    wc2 = np.concatenate(
        [np.asarray(W2, np.float32),
         (W2_64 @ np.asarray(a2_src[0], np.float64))[:, None].astype(np.float32),
         (W2_64 @ np.asarray(a2_dst[0], np.float64))[:, None].astype(np.float32)],
        1)                                               # [F, OUT+2]

    KT = F // 128
    xt = np.zeros((CORES, KT, 128, NPCP), np.float32)
    xp = np.asarray(x, np.float32)[perm]
    for c in range(CORES):
        xt[c, :, :, :NPC] = (xp[c * NPC:(c + 1) * NPC].T
                             .reshape(KT, 128, NPC))
    # bf16 inputs for the phase-A matmuls (PE accumulates in f32)
    import ml_dtypes
    xt_bf = xt.astype(ml_dtypes.bfloat16)
    wc1_bf = wc1.astype(ml_dtypes.bfloat16)

    host = {
        "N": N, "F": F, "H": H, "C": C, "OUT": OUT,
        "NPC": NPC, "W": W, "NPCP": NPCP, "NQ": NQ, "QH": QH, "WQ": WQ,
        "K": tuple(int(k) for k in K), "SK": SK, "EPC": EPC,
        "perm": perm,
        "use_b1": bool(np.any(b1)), "use_b2": bool(np.any(b2)),
    }
    in_maps = []
    for c in range(CORES):
        m = {
            "xt": xt_bf[c],
            "wc1": np.ascontiguousarray(wc1_bf.reshape(2, F // 2, F + 2 * H)),
            "wc2": np.ascontiguousarray(wc2.reshape(2, F // 2, OUT + 2)),
            "isrc": isrc_w[c],
            "isrc2": isrc2_w[c],
        }
        if host["use_b1"]:
            m["b1rep"] = np.broadcast_to(np.asarray(b1, np.float32), (PW, F)).copy()
        if host["use_b2"]:
            m["b2rep"] = np.broadcast_to(np.asarray(b2, np.float32), (PW, OUT)).copy()
        in_maps.append(m)
    return host, in_maps


# --------------------------------------------------------------------------
# device kernel
# --------------------------------------------------------------------------

def _build(hp):
    F, H, C, OUT = hp["F"], hp["H"], hp["C"], hp["OUT"]
    W, NPCP = hp["W"], hp["NPCP"]
    NQ, QH, WQ = hp["NQ"], hp["QH"], hp["WQ"]
    K, SK, EPC = hp["K"], hp["SK"], hp["EPC"]
    KT = F // 128               # K tiles (2)
    # layer-1 record, 512 bytes: [256 x fp8 features | 16 x f32 alphas | pad]
    REC1 = 512                  # fp8 (byte) units
    assert REC1 % 256 == 0 and F + 64 <= REC1
    A1 = F + 64                 # valid bytes (features + alpha area)
    AF = F + 2 * H              # 272 f32 matmul output cols
    REC2 = 128                  # bf16 units = 256 B rec2 rows
    A2 = OUT + 4                # valid bf16 cols: 40 h2 + 2 f32 alphas
    A2M = OUT + 2               # f32 matmul output cols of wc2
    NTAB = CORES * NPCP
    EPC16 = EPC // 16
    NPC = hp["NPC"]
    CK = [0]
    for k in K:
        CK.append(CK[-1] + k)

    # window processing order (largest degree first) and groups of GRP
    order = list(reversed(range(W)))
    groups = [order[i:i + GRP] for i in range(0, W, GRP)]

    nc = bacc.Bacc(None, target_bir_lowering=False, num_swdge_queues=GRP)

    xt_p = nc.declare_dram_parameter("xt", [KT, 128, NPCP], BF16, isOutput=False)
    wc1_p = nc.declare_dram_parameter("wc1", [KT, 128, AF], BF16, isOutput=False)
    wc2_p = nc.declare_dram_parameter("wc2", [KT, 128, A2M], F32, isOutput=False)
    isrc_p = nc.declare_dram_parameter("isrc", [128, EPC16], I16, isOutput=False)
    isrc2_p = nc.declare_dram_parameter("isrc2", [128, EPC16], I16, isOutput=False)
    b1_p = (nc.declare_dram_parameter("b1rep", [PW, F], F32, isOutput=False)
            if hp["use_b1"] else None)
    b2_p = (nc.declare_dram_parameter("b2rep", [PW, OUT], F32, isOutput=False)
            if hp["use_b2"] else None)
    out_p = nc.declare_dram_parameter("out", [NPCP, OUT], F32, isOutput=True)

    with tile.TileContext(nc) as tc:
        with (
            tc.tile_pool(name="dram", bufs=1, space="DRAM") as dram,
            tc.tile_pool(name="const", bufs=1) as cpool,
            tc.tile_pool(name="gath", bufs=3) as gp,
            tc.tile_pool(name="mid", bufs=2) as mp,
            tc.tile_pool(name="psA", bufs=4, space="PSUM") as psA,
            tc.tile_pool(name="psB", bufs=2, space="PSUM") as psB,
        ):
            r1loc = dram.tile([NPCP, REC1], FP8)
            tab1 = dram.tile([NTAB, REC1], FP8, addr_space="Shared")
            r2loc = dram.tile([NPCP, REC2], BF16)
            tab2 = dram.tile([NTAB, REC2], BF16, addr_space="Local")

            # resident constants
            wc1_sb = cpool.tile([128, KT, AF], BF16)
            for g in range(KT):
                nc.sync.dma_start(out=wc1_sb[:, g, :], in_=wc1_p[g])
            xt_sb = cpool.tile([128, KT, NPCP], BF16)
            for g in range(KT):
                nc.sync.dma_start(out=xt_sb[:, g, :], in_=xt_p[g])
            wc2_sb = cpool.tile([128, KT, A2M], F32)
            for g in range(KT):
                nc.sync.dma_start(out=wc2_sb[:, g, :], in_=wc2_p[g])
            ident = cpool.tile([PW, PW], F32)
            make_identity(nc, ident[:])
            isrc_sb = cpool.tile([128, EPC16], I16)
            nc.sync.dma_start(out=isrc_sb[:], in_=isrc_p[:])
            isrc2_sb = cpool.tile([128, EPC16], I16)
            nc.sync.dma_start(out=isrc2_sb[:], in_=isrc2_p[:])
            if b1_p is not None:
                b1_sb = cpool.tile([PW, F], F32)
                nc.sync.dma_start(out=b1_sb[:], in_=b1_p[:])
            if b2_p is not None:
                b2_sb = cpool.tile([PW, OUT], F32)
                nc.sync.dma_start(out=b2_sb[:], in_=b2_p[:])
            pcA = cpool.tile([PW, 16], F32)      # pad-row alphas for rec1
            nc.vector.memset(pcA[:], 0.0)
            nc.vector.memset(pcA[:, :H], -200.0)
            pzA = cpool.tile([PW, F], FP8)       # pad-row features (zero)
            nc.vector.memset(pzA[:], 0.0)
            pcB = cpool.tile([PW, 2], F32)       # pad-row alphas for rec2
            nc.vector.memset(pcB[:], 0.0)
            nc.vector.memset(pcB[:, :1], -200.0)
            pzB = cpool.tile([PW, OUT], BF16)
            nc.vector.memset(pzB[:], 0.0)
            lgall = cpool.tile([128, W, OUT], F32)
            ezsc = cpool.tile([128, OUT], F32)
            ssall = cpool.tile([128, W], F32)
            lsall = cpool.tile([128, W], F32)

            # ---------------- phase A: rec1 rows for this core ------------
            # (gathered pad columns [A1:REC1) / [A2:REC2) are never read, so
            # the tables are left uninitialized there)
            for nt in range(W):
                rp = psB.tile([128, AF], F32, tag="acc")
                for g in range(KT):
                    nc.tensor.matmul(rp[:], lhsT=xt_sb[:, g, nt * PW:(nt + 1) * PW],
                                     rhs=wc1_sb[:, g, :],
                                     start=(g == 0), stop=(g == KT - 1))
                rs = mp.tile([128, A1], FP8, tag="rs")
                nc.vector.tensor_copy(out=rs[:, F:F + 64].bitcast(F32),
                                      in_=rp[:, F:])
                if b1_p is not None:
                    # bias folded into the aggregated features (sum alpha = 1)
                    nc.vector.tensor_add(out=rs[:, :F], in0=rp[:, :F], in1=b1_sb[:])
                else:
                    nc.vector.tensor_copy(out=rs[:, :F], in_=rp[:, :F])
                nc.sync.dma_start(out=r1loc[nt * PW:(nt + 1) * PW, :A1], in_=rs[:])
                if nt == W - 1:
                    # pad rows: zero features, alpha_src=-200 (exp ~ 4e-18)
                    for lo in range(NPC, NPCP, PW):
                        nr = min(PW, NPCP - lo)
                        nc.sync.dma_start(out=r1loc[lo:lo + nr, :F],
                                          in_=pzA[:nr])
                        nc.sync.dma_start(
                            out=r1loc[lo:lo + nr, F:F + 64].bitcast(F32),
                            in_=pcA[:nr])
                    # single AllGather: per-op fixed cost (~25us) makes
                    # quarter-split collectives a net loss for this table.
                    nc.gpsimd.collective_compute(
                        "AllGather", ALU.bypass,
                        replica_groups=[list(range(CORES))],
                        ins=[r1loc[:, :].opt()],
                        outs=[tab1[:, :].opt()])

            # ---------------- phase B: layer-1 edge phase -----------------
            cc2_fired = [False] * NQ

            def fire_cc2(done_min):
                for q in range(NQ - 1, -1, -1):
                    if cc2_fired[q] or done_min > q * WQ:
                        continue
                    cc2_fired[q] = True
                    nc.gpsimd.collective_compute(
                        "AllGather", ALU.bypass,
                        replica_groups=[list(range(CORES))],
                        ins=[r2loc[q * QH:(q + 1) * QH, :].opt()],
                        outs=[tab2[q * CORES * QH:(q + 1) * CORES * QH, :].opt()])

            # self-loop attention terms for all windows in one shot:
            # exp(leaky_relu(a_s + a_d)) on [128, W, H]
            aall = cpool.tile([128, W, 16], F32)
            nc.sync.dma_start(
                out=aall[:],
                in_=r1loc[:W * PW, F:F + 64].bitcast(F32)
                    .rearrange("(w p) a -> p w a", p=PW))
            essall = cpool.tile([128, W, H], F32)
            nc.vector.tensor_tensor(out=essall[:], in0=aall[:, :, :H],
                                    in1=aall[:, :, H:], op=ALU.add)
            nc.vector.scalar_tensor_tensor(
                out=essall[:], in0=essall[:], scalar=NEG_SLOPE, in1=essall[:],
                op0=ALU.mult, op1=ALU.max)
            nc.scalar.activation(out=essall[:], in_=essall[:], func=ACTF.Exp)

            for w in order:
                kw = K[w]
                # the window's gather is split over the 4 SWDGE queues
                # (disjoint column ranges of one tile): descriptor generation
                # runs on all 4 Q7 core pairs concurrently (~3x), and the
                # per-window ring population keeps transfers window-serial so
                # consumers pipeline behind them.
                G1 = gp.tile([128, kw, REC1], FP8, tag="G1")
                qof = [kw * q // GRP for q in range(GRP + 1)]
                for q in range(GRP):
                    kq = qof[q + 1] - qof[q]
                    if kq == 0:
                        continue
                    nc.gpsimd.dma_gather(
                        G1[:, qof[q]:qof[q + 1], :], tab1[:, :],
                        isrc_sb[:, (CK[w] + qof[q]) * 8:(CK[w] + qof[q + 1]) * 8],
                        kq * PW, kq * PW, REC1, single_packet=False,
                        queue_num=q)
                if True:
                    loc = mp.tile([128, F], FP8, tag="loc")
                    nc.sync.dma_start(out=loc[:],
                                      in_=r1loc[w * PW:(w + 1) * PW, :F])
                    # ex = exp(leaky_relu(a_s[src] + a_d[dst]))  (pads -> ~0)
                    es_t = mp.tile([128, kw, H], F32, tag="es")
                    es = es_t[:, :, :]
                    nc.vector.tensor_tensor(
                        out=es, in0=G1[:, :, F:F + 32].bitcast(F32),
                        in1=aall[:, w, H:].unsqueeze(1)
                            .to_broadcast([128, kw, H]),
                        op=ALU.add)
                    nc.vector.scalar_tensor_tensor(
                        out=es, in0=es, scalar=NEG_SLOPE, in1=es,
                        op0=ALU.mult, op1=ALU.max)
                    nc.scalar.activation(out=es, in_=es, func=ACTF.Exp)
                    # alpha = ex / (sum_t ex + ex_self), folded to bf16 so the
                    # big multiply runs at the DVE 16-bit rate
                    den = mp.tile([128, H], F32, tag="den")
                    nc.vector.tensor_reduce(
                        out=den[:], in_=es.rearrange("p t h -> p h t"),
                        axis=AXX, op=ALU.add)
                    nc.vector.tensor_add(out=den[:], in0=den[:],
                                         in1=essall[:, w, :])
                    rcp = mp.tile([128, H], F32, tag="rcp")
                    nc.vector.reciprocal(rcp[:], den[:])
                    abf = mp.tile([128, kw, H], BF16, tag="abf")
                    nc.vector.tensor_tensor(
                        out=abf[:], in0=es,
                        in1=rcp[:].unsqueeze(1).to_broadcast([128, kw, H]),
                        op=ALU.mult)
                    asf = mp.tile([128, H], F32, tag="asf")
                    nc.vector.tensor_tensor(out=asf[:], in0=essall[:, w, :],
                                            in1=rcp[:], op=ALU.mult)
                    # weighted feature sum over edge columns + self
                    GW = mp.tile([128, kw, F], BF16, tag="GW")
                    nc.vector.tensor_tensor(
                        out=GW[:].rearrange("p t (h c) -> p t h c", h=H),
                        in0=G1[:, :, :F].rearrange("p t (h c) -> p t h c", h=H),
                        in1=abf[:].unsqueeze(3).to_broadcast([128, kw, H, C]),
                        op=ALU.mult)
                    # pairwise-tree segment sum over t: contiguous bf16 adds
                    # run at the DVE 16-bit rate (the strided f32 tensor_reduce
                    # over the t axis is ~2.5x slower)
                    n = kw
                    while n > 1:
                        if n % 2 == 1:
                            nc.vector.tensor_tensor(
                                out=GW[:, 0, :], in0=GW[:, 0, :],
                                in1=GW[:, n - 1, :], op=ALU.add)
                        h2 = n // 2
                        nc.vector.tensor_tensor(
                            out=GW[:, :h2, :], in0=GW[:, :h2, :],
                            in1=GW[:, h2:2 * h2, :], op=ALU.add)
                        n = h2
                    tmp = mp.tile([128, F], F32, tag="tmp")
                    nc.vector.tensor_tensor(
                        out=tmp[:].rearrange("p (h c) -> p h c", h=H),
                        in0=loc[:].rearrange("p (h c) -> p h c", h=H),
                        in1=asf[:].unsqueeze(2).to_broadcast([128, H, C]),
                        op=ALU.mult)
                    ho = mp.tile([128, F], F32, tag="ho")
                    nc.vector.tensor_add(out=ho[:], in0=GW[:, 0, :], in1=tmp[:])
                    # ELU(x) = relu(x) + exp(-relu(-x)) - 1, with all three
                    # activations on the (otherwise idle) scalar engine
                    xm = mp.tile([128, F], F32, tag="xm")
                    nc.scalar.activation(out=xm[:], in_=ho[:], func=ACTF.Relu,
                                         scale=-1.0)
                    nc.scalar.activation(out=xm[:], in_=xm[:], func=ACTF.Exp,
                                         scale=-1.0)
                    nc.scalar.activation(out=ho[:], in_=ho[:], func=ACTF.Relu)
                    nc.vector.scalar_tensor_tensor(
                        out=ho[:], in0=ho[:], scalar=-1.0, in1=xm[:],
                        op0=ALU.add, op1=ALU.add)
                    hT = mp.tile([128, KT, 128], F32, tag="hT")
                    for g in range(KT):
                        tp = psA.tile([128, 128], F32, tag="tp")
                        nc.tensor.transpose(out=tp[:],
                                            in_=ho[:, g * 128:(g + 1) * 128],
                                            identity=ident[:])
                        nc.vector.tensor_copy(out=hT[:, g, :], in_=tp[:])
                    r2p = psB.tile([128, A2M], F32, tag="acc2")
                    for g in range(KT):
                        nc.tensor.matmul(r2p[:], lhsT=hT[:, g, :],
                                         rhs=wc2_sb[:, g, :],
                                         start=(g == 0), stop=(g == KT - 1))
                    r2sb = mp.tile([128, A2], BF16, tag="r2sb")
                    nc.vector.tensor_copy(out=r2sb[:, :OUT], in_=r2p[:, :OUT])
                    nc.vector.tensor_copy(
                        out=r2sb[:, OUT:OUT + 4].bitcast(F32),
                        in_=r2p[:, OUT:OUT + 2])
                    nc.sync.dma_start(out=r2loc[w * PW:(w + 1) * PW, :A2],
                                      in_=r2sb[:])
                    if w == W - 1 and NPCP > NPC:
                        # pad rows (inside the last quarter's row range)
                        for lo in range(NPC, NPCP, PW):
                            nr = min(PW, NPCP - lo)
                            nc.sync.dma_start(out=r2loc[lo:lo + nr, :OUT],
                                              in_=pzB[:nr])
                            nc.sync.dma_start(
                                out=r2loc[lo:lo + nr, OUT:OUT + 4].bitcast(F32),
                                in_=pcB[:nr])
                fire_cc2(w)

            # ---------------- phase D: layer-2 edge phase -----------------
            # batched self-loop attention terms from the rec2 alpha columns
            a2all = cpool.tile([128, W, 2], F32)
            nc.sync.dma_start(
                out=a2all[:],
                in_=r2loc[:W * PW, OUT:OUT + 4].bitcast(F32)
                    .rearrange("(w p) a -> p w a", p=PW))
            ess2all = cpool.tile([128, W], F32)
            nc.vector.tensor_tensor(out=ess2all[:], in0=a2all[:, :, 0],
                                    in1=a2all[:, :, 1], op=ALU.add)
            nc.vector.scalar_tensor_tensor(
                out=ess2all[:], in0=ess2all[:], scalar=NEG_SLOPE,
                in1=ess2all[:], op0=ALU.mult, op1=ALU.max)
            nc.scalar.activation(out=ess2all[:], in_=ess2all[:], func=ACTF.Exp)

            for w in order:
                kw = K[w]
                G2 = gp.tile([128, kw, REC2], BF16, tag="G2")
                qof = [kw * q // GRP for q in range(GRP + 1)]
                for q in range(GRP):
                    kq = qof[q + 1] - qof[q]
                    if kq == 0:
                        continue
                    nc.gpsimd.dma_gather(
                        G2[:, qof[q]:qof[q + 1], :], tab2[:, :],
                        isrc2_sb[:, (CK[w] + qof[q]) * 8:(CK[w] + qof[q + 1]) * 8],
                        kq * PW, kq * PW, REC2, single_packet=False,
                        queue_num=q)
                if True:
                    loc2 = mp.tile([128, OUT], BF16, tag="loc2")
                    nc.sync.dma_start(out=loc2[:],
                                      in_=r2loc[w * PW:(w + 1) * PW, :OUT])
                    es2_t = mp.tile([128, kw, 1], F32, tag="es2")
                    es2 = es2_t[:, :, :]
                    nc.vector.tensor_tensor(
                        out=es2, in0=G2[:, :, OUT:OUT + 2].bitcast(F32),
                        in1=a2all[:, w, 1:2].unsqueeze(1)
                            .to_broadcast([128, kw, 1]),
                        op=ALU.add)
                    nc.vector.scalar_tensor_tensor(
                        out=es2, in0=es2, scalar=NEG_SLOPE, in1=es2,
                        op0=ALU.mult, op1=ALU.max)
                    nc.scalar.activation(out=es2, in_=es2, func=ACTF.Exp)
                    den2 = mp.tile([128, 1], F32, tag="den2")
                    nc.vector.tensor_reduce(
                        out=den2[:], in_=es2.rearrange("p t h -> p h t"),
                        axis=AXX, op=ALU.add)
                    nc.vector.tensor_add(out=den2[:], in0=den2[:],
                                         in1=ess2all[:, w:w + 1])
                    rcp2 = mp.tile([128, 1], F32, tag="rcp2")
                    nc.vector.reciprocal(rcp2[:], den2[:])
                    nc.vector.tensor_scalar_mul(out=es2, in0=es2,
                                                scalar1=rcp2[:, :1])
                    nc.vector.tensor_tensor(
                        out=G2[:, :, :OUT], in0=G2[:, :, :OUT],
                        in1=es2.to_broadcast([128, kw, OUT]), op=ALU.mult)
                    num2 = mp.tile([128, OUT], F32, tag="num2")
                    nc.vector.tensor_reduce(
                        out=num2[:], in_=G2[:, :, :OUT].rearrange("p t f -> p f t"),
                        axis=AXX, op=ALU.add)
                    tmp2 = mp.tile([128, OUT], F32, tag="tmp2")
                    nc.vector.tensor_scalar(
                        out=tmp2[:], in0=loc2[:], scalar1=ess2all[:, w:w + 1],
                        scalar2=rcp2[:, :1], op0=ALU.mult, op1=ALU.mult)
                    nc.vector.tensor_add(out=lgall[:, w, :], in0=num2[:],
                                         in1=tmp2[:])
                    if b2_p is not None:
                        nc.vector.tensor_add(out=lgall[:, w, :],
                                             in0=lgall[:, w, :], in1=b2_sb[:])

            # batched log_softmax over all windows (no max-shift: logits are
            # bounded ~[-2, 2]); one ACT table load for the W Exps, one Ln.
            for w in range(W):
                nc.scalar.activation(out=ezsc[:], in_=lgall[:, w, :],
                                     func=ACTF.Exp, accum_out=ssall[:, w:w + 1])
            nc.scalar.activation(out=lsall[:], in_=ssall[:], func=ACTF.Ln)
            nc.vector.tensor_tensor(
                out=lgall[:], in0=lgall[:],
                in1=lsall[:].unsqueeze(2).to_broadcast([128, W, OUT]),
                op=ALU.subtract)
            nc.sync.dma_start(
                out=out_p[:W * PW].rearrange("(w p) o -> p w o", p=PW),
                in_=lgall[:])

    nc.compile()
    return nc


# --------------------------------------------------------------------------
# public entry point
# --------------------------------------------------------------------------

def kernel(x, edge_index, W1, a1_src, a1_dst, b1, W2, a2_src, a2_dst, b2,
           _want_trace=False):
    x = np.asarray(x)
    host, in_maps = _prep(x, np.asarray(edge_index), np.asarray(W1),
                          np.asarray(a1_src), np.asarray(a1_dst),
                          np.asarray(b1), np.asarray(W2), np.asarray(a2_src),
                          np.asarray(a2_dst), np.asarray(b2))
    key = (host["N"], host["F"], host["H"], host["C"], host["OUT"],
           host["K"], host["NQ"], host["use_b1"], host["use_b2"])
    if key not in _CACHE:
        _CACHE[key] = _build(host)
    nc = _CACHE[key]
    res = run_bass_kernel_spmd(nc, in_maps, core_ids=list(range(CORES)),
                               trace=_want_trace)
    NPC = host["NPC"]
    out = np.empty((host["N"], host["OUT"]), np.float32)
    for c in range(CORES):
        out[host["perm"][c * NPC:(c + 1) * NPC]] = res.results[c]["out"][:NPC]
    if _want_trace:
        kernel._last_result = res
    return np.ascontiguousarray(out)


# revision 58
# speedup vs baseline: 1.1412x; 1.0264x over previous
"""Two-layer GAT (PyG-style, eval mode) on 8 Trainium2 NeuronCores.

Strategy (dst-sharded, per-node edge columns):
  - Host: shard destination nodes into 8 contiguous ranges (one per core).
    Within a core, nodes are permuted by in-degree so that each window of 128
    nodes has near-uniform degree; window w stores its edges as K[w] columns
    of 128 rows, where row p holds the in-edges of the window's node p
    (padded to K[w] = max in-degree in the window, ~12% overhead).
    Self-loops are excluded from the edge lists (their features are local).
  - Device phase A: per-core rows of  rec1 = [x @ W1 + b1 | alpha_src | alpha_dst]
    via PE matmuls (alpha vectors folded into the weight matrix); the rec1
    AllGather is split into 4 quarter-table collectives, each fired as soon
    as its 5 windows of rows are computed, so the table exchange pipelines
    behind phase A.
  - Device phase B (layer-1 edge phase): windows are processed in GROUPS OF
    4; the 4 gathers of a group are issued back-to-back on SWDGE queues 0-3,
    whose descriptor generation runs on disjoint Q7 core pairs and therefore
    overlaps ~3x.  A small DVE copy stages each group's gather indices, which
    serializes the group's desc-gen burst after the previous group's vector
    work -- without this the gathers spread out uniformly and the DVE (which
    stalls hard while any SWDGE desc-gen is active: shared SBUF port) crawls
    through the whole phase.  Within a window: ex = exp(leaky_relu(a_s+a_d))
    without the segment-max shift (edge logits are bounded ~[-2,2], so this
    is numerically exact); pad edge columns gather a dedicated pad row whose
    alpha_src is -200 => exp ~ 4e-18, no masking.  alpha = ex/den is folded
    to bf16 BEFORE the big weighted multiply so the multiply runs at the
    DVE's 2x 16-bit rate; the segment sum is one strided tensor_reduce.
    rec2 = [h @ W2 | alpha2_src | alpha2_dst] rows stream out per window and
    the rec2 AllGather fires in quarters as their windows complete.
  - Phase D repeats the grouped edge phase for layer 2 (1 head, 40 ch),
    buffering per-window logits on chip; log_softmax runs as one batched
    tail pass.  The host inverse-permutes the rows.

    Layer-1 records are 768 B: 256 bf16 features + 16 fp32 alphas + pad
    (gather elements must be 256B-multiples).  Measured bottlenecks: SWDGE
    descriptor emission (~8.7 ns per gathered row serial, ~2.8x when 4
    queues emit concurrently) and the DVE stall during desc-gen.
"""

import sys

for _p in ("/opt/trn_rl_repo", "/root/.axon_site/_ro/trn_rl_repo"):
    if _p not in sys.path:
        sys.path.append(_p)

import numpy as np

import concourse.bass as bass
import concourse.mybir as mybir
import concourse.tile as tile
from concourse import bacc
from concourse.bass_utils import run_bass_kernel_spmd
from concourse.masks import make_identity

F32 = mybir.dt.float32
I16 = mybir.dt.int16
BF16 = mybir.dt.bfloat16
FP8 = mybir.dt.float8e4
ALU = mybir.AluOpType
ACTF = mybir.ActivationFunctionType
AXX = mybir.AxisListType.X

CORES = 8
PW = 128           # nodes per window
NEG_SLOPE = 0.2
GRP = 4            # windows per gather group == SWDGE queues used

_CACHE = {}


# --------------------------------------------------------------------------
# host-side preprocessing
# --------------------------------------------------------------------------

def _wrap_idx(vals):
    """Wrap a flat index vector into the [128, n/16] layout dma_gather wants
    (index i at partition i%16, col i//16; replicated to all 8 Q7 groups)."""
    w = vals.reshape(-1, 16).T.astype(np.int16)
    return np.ascontiguousarray(np.tile(w, (8, 1)))


def _prep(x, edge_index, W1, a1_src, a1_dst, b1, W2, a2_src, a2_dst, b2):
    N, F = x.shape
    H, C = a1_src.shape
    OUT = W2.shape[1]
    NPC = N // CORES
    W = (NPC + PW - 1) // PW
    NPCP = W * PW
    if NPCP == NPC:
        NPCP += PW          # spare pad block (holds the -200 pad row)
        W += 1

    # quarter-split of the node tables: NQ contiguous row blocks per core,
    # stored [quarter, core, QH] so each AllGather's output is contiguous.
    NQ = 4 if W % 4 == 0 else (2 if W % 2 == 0 else 1)
    WQ = W // NQ
    QH = WQ * PW
    assert NPC > (NQ - 1) * QH   # pad rows live in the last quarter

    src0 = np.asarray(edge_index[0], np.int64)
    dst0 = np.asarray(edge_index[1], np.int64)

    deg = np.zeros(N, np.int64)
    np.add.at(deg, dst0, 1)

    # per-core degree-sorted node permutation; perm[new_global_row] = node id
    perm = np.empty(N, np.int64)
    for c in range(CORES):
        ids = np.arange(c * NPC, (c + 1) * NPC)
        perm[c * NPC:(c + 1) * NPC] = ids[np.argsort(deg[ids], kind="stable")]
    newrow = np.empty(N, np.int64)
    newrow[perm] = np.arange(N)          # node id -> permuted global row
    # tab1 rows: [core, NPCP] (single AllGather); tab2 rows: [quarter, core,
    # QH] so its AllGather can fire in quarters as windows complete.
    _c = newrow // NPC
    _r = newrow % NPC
    tabrow = _c * NPCP + _r
    tabrow2 = (_r // QH) * (CORES * QH) + _c * QH + (_r % QH)

    # per-window max degree (shared across cores for SPMD uniformity)
    degw = deg[perm].reshape(CORES, NPC)
    K = []
    for w in range(W):
        lo, hi = w * PW, min((w + 1) * PW, NPC)
        if lo >= hi:
            K.append(1)
        else:
            K.append(max(1, int(degw[:, lo:hi].max())))
    K = np.asarray(K, np.int64)
    CK = np.concatenate([[0], np.cumsum(K)])   # column offsets
    SK = int(K.sum())
    EPC = SK * PW                              # gather indices per core/layer

    # in-edges grouped by (permuted) destination row: CSR over new rows
    order = np.argsort(newrow[dst0], kind="stable")
    src_s = src0[order]
    starts = np.zeros(N + 1, np.int64)
    np.add.at(starts, newrow[dst0] + 1, 1)
    starts = np.cumsum(starts)

    pq, po = NPC // QH, NPC % QH               # pad row quarter/offset
    isrc = np.zeros((CORES, EPC), np.int64)
    isrc2 = np.zeros((CORES, EPC), np.int64)
    for c in range(CORES):
        # pads gather the core's first pad row (alpha_src overwritten to -200)
        isrc[c, :] = c * NPCP + NPC
        isrc2[c, :] = pq * (CORES * QH) + c * QH + po
        for w in range(W):
            kw = int(K[w])
            for p in range(PW):
                lp = w * PW + p
                if lp >= NPC:
                    continue
                r = c * NPC + lp
                s, e = starts[r], starts[r + 1]
                d = e - s
                cols = CK[w] + np.arange(d)
                isrc[c, cols * PW + p] = tabrow[src_s[s:e]]
                isrc2[c, cols * PW + p] = tabrow2[src_s[s:e]]

    isrc_w = np.stack([_wrap_idx(isrc[c]) for c in range(CORES)])
    isrc2_w = np.stack([_wrap_idx(isrc2[c]) for c in range(CORES)])

    # folded weight matrices (f64 for exactness of the tiny folds)
    As = np.zeros((F, H), np.float64)
    Ad = np.zeros((F, H), np.float64)
    for h in range(H):
        As[h * C:(h + 1) * C, h] = a1_src[h]
        Ad[h * C:(h + 1) * C, h] = a1_dst[h]
    W1_64 = np.asarray(W1, np.float64)
    wc1 = np.concatenate(
        [np.asarray(W1, np.float32),
         (W1_64 @ As).astype(np.float32),
         (W1_64 @ Ad).astype(np.float32)], 1)           # [F, F+2H]
    W2_64 = np.asarray(W2, np.float64)
    wc2 = np.concatenate(
        [np.asarray(W2, np.float32),
         (W2_64 @ np.asarray(a2_src[0], np.float64))[:, None].astype(np.float32),
         (W2_64 @ np.asarray(a2_dst[0], np.float64))[:, None].astype(np.float32)],
        1)                                               # [F, OUT+2]

    KT = F // 128
    xt = np.zeros((CORES, KT, 128, NPCP), np.float32)
    xp = np.asarray(x, np.float32)[perm]
    for c in range(CORES):
        xt[c, :, :, :NPC] = (xp[c * NPC:(c + 1) * NPC].T
                             .reshape(KT, 128, NPC))
    # bf16 inputs for the phase-A matmuls (PE accumulates in f32)
    import ml_dtypes
    xt_bf = xt.astype(ml_dtypes.bfloat16)
    wc1_bf = wc1.astype(ml_dtypes.bfloat16)

    host = {
        "N": N, "F": F, "H": H, "C": C, "OUT": OUT,
        "NPC": NPC, "W": W, "NPCP": NPCP, "NQ": NQ, "QH": QH, "WQ": WQ,
        "K": tuple(int(k) for k in K), "SK": SK, "EPC": EPC,
        "perm": perm,
        "use_b1": bool(np.any(b1)), "use_b2": bool(np.any(b2)),
    }
    in_maps = []
    for c in range(CORES):
        m = {
            "xt": xt_bf[c],
            "wc1": np.ascontiguousarray(wc1_bf.reshape(2, F // 2, F + 2 * H)),
            "wc2": np.ascontiguousarray(wc2.reshape(2, F // 2, OUT + 2)),
            "isrc": isrc_w[c],
            "isrc2": isrc2_w[c],
        }
        if host["use_b1"]:
            m["b1rep"] = np.broadcast_to(np.asarray(b1, np.float32), (PW, F)).copy()
        if host["use_b2"]:
            m["b2rep"] = np.broadcast_to(np.asarray(b2, np.float32), (PW, OUT)).copy()
        in_maps.append(m)
    return host, in_maps


# --------------------------------------------------------------------------
# device kernel
# --------------------------------------------------------------------------

def _build(hp):
    F, H, C, OUT = hp["F"], hp["H"], hp["C"], hp["OUT"]
    W, NPCP = hp["W"], hp["NPCP"]
    NQ, QH, WQ = hp["NQ"], hp["QH"], hp["WQ"]
    K, SK, EPC = hp["K"], hp["SK"], hp["EPC"]
    KT = F // 128               # K tiles (2)
    # layer-1 record, 512 bytes: [256 x fp8 features | 16 x f32 alphas | pad]
    REC1 = 512                  # fp8 (byte) units
    assert REC1 % 256 == 0 and F + 64 <= REC1
    A1 = F + 64                 # valid bytes (features + alpha area)
    AF = F + 2 * H              # 272 f32 matmul output cols
    REC2 = 128                  # bf16 units = 256 B rec2 rows
    A2 = OUT + 4                # valid bf16 cols: 40 h2 + 2 f32 alphas
    A2M = OUT + 2               # f32 matmul output cols of wc2
    NTAB = CORES * NPCP
    EPC16 = EPC // 16
    NPC = hp["NPC"]
    CK = [0]
    for k in K:
        CK.append(CK[-1] + k)

    # window processing order (largest degree first) and groups of GRP
    order = list(reversed(range(W)))
    groups = [order[i:i + GRP] for i in range(0, W, GRP)]

    nc = bacc.Bacc(None, target_bir_lowering=False, num_swdge_queues=GRP)

    xt_p = nc.declare_dram_parameter("xt", [KT, 128, NPCP], BF16, isOutput=False)
    wc1_p = nc.declare_dram_parameter("wc1", [KT, 128, AF], BF16, isOutput=False)
    wc2_p = nc.declare_dram_parameter("wc2", [KT, 128, A2M], F32, isOutput=False)
    isrc_p = nc.declare_dram_parameter("isrc", [128, EPC16], I16, isOutput=False)
    isrc2_p = nc.declare_dram_parameter("isrc2", [128, EPC16], I16, isOutput=False)
    b1_p = (nc.declare_dram_parameter("b1rep", [PW, F], F32, isOutput=False)
            if hp["use_b1"] else None)
    b2_p = (nc.declare_dram_parameter("b2rep", [PW, OUT], F32, isOutput=False)
            if hp["use_b2"] else None)
    out_p = nc.declare_dram_parameter("out", [NPCP, OUT], F32, isOutput=True)

    with tile.TileContext(nc) as tc:
        with (
            tc.tile_pool(name="dram", bufs=1, space="DRAM") as dram,
            tc.tile_pool(name="const", bufs=1) as cpool,
            tc.tile_pool(name="gath", bufs=4) as gp,
            tc.tile_pool(name="mid", bufs=2) as mp,
            tc.tile_pool(name="psA", bufs=4, space="PSUM") as psA,
            tc.tile_pool(name="psB", bufs=2, space="PSUM") as psB,
        ):
            r1loc = dram.tile([NPCP, REC1], FP8)
            tab1 = dram.tile([NTAB, REC1], FP8, addr_space="Shared")
            r2loc = dram.tile([NPCP, REC2], BF16)
            tab2 = dram.tile([NTAB, REC2], BF16, addr_space="Local")

            # resident constants
            wc1_sb = cpool.tile([128, KT, AF], BF16)
            for g in range(KT):
                nc.sync.dma_start(out=wc1_sb[:, g, :], in_=wc1_p[g])
            xt_sb = cpool.tile([128, KT, NPCP], BF16)
            for g in range(KT):
                nc.sync.dma_start(out=xt_sb[:, g, :], in_=xt_p[g])
            wc2_sb = cpool.tile([128, KT, A2M], F32)
            for g in range(KT):
                nc.sync.dma_start(out=wc2_sb[:, g, :], in_=wc2_p[g])
            ident = cpool.tile([PW, PW], F32)
            make_identity(nc, ident[:])
            isrc_sb = cpool.tile([128, EPC16], I16)
            nc.sync.dma_start(out=isrc_sb[:], in_=isrc_p[:])
            isrc2_sb = cpool.tile([128, EPC16], I16)
            nc.sync.dma_start(out=isrc2_sb[:], in_=isrc2_p[:])
            if b1_p is not None:
                b1_sb = cpool.tile([PW, F], F32)
                nc.sync.dma_start(out=b1_sb[:], in_=b1_p[:])
            if b2_p is not None:
                b2_sb = cpool.tile([PW, OUT], F32)
                nc.sync.dma_start(out=b2_sb[:], in_=b2_p[:])
            pcA = cpool.tile([PW, 16], F32)      # pad-row alphas for rec1
            nc.vector.memset(pcA[:], 0.0)
            nc.vector.memset(pcA[:, :H], -200.0)
            pzA = cpool.tile([PW, F], FP8)       # pad-row features (zero)
            nc.vector.memset(pzA[:], 0.0)
            pcB = cpool.tile([PW, 2], F32)       # pad-row alphas for rec2
            nc.vector.memset(pcB[:], 0.0)
            nc.vector.memset(pcB[:, :1], -200.0)
            pzB = cpool.tile([PW, OUT], BF16)
            nc.vector.memset(pzB[:], 0.0)
            lgall = cpool.tile([128, W, OUT], F32)
            ezsc = cpool.tile([128, OUT], F32)
            ssall = cpool.tile([128, W], F32)
            lsall = cpool.tile([128, W], F32)

            # ---------------- phase A: rec1 rows for this core ------------
            # (gathered pad columns [A1:REC1) / [A2:REC2) are never read, so
            # the tables are left uninitialized there)
            for nt in range(W):
                rp = psB.tile([128, AF], F32, tag="acc")
                for g in range(KT):
                    nc.tensor.matmul(rp[:], lhsT=xt_sb[:, g, nt * PW:(nt + 1) * PW],
                                     rhs=wc1_sb[:, g, :],
                                     start=(g == 0), stop=(g == KT - 1))
                rs = mp.tile([128, A1], FP8, tag="rs")
                nc.vector.tensor_copy(out=rs[:, F:F + 64].bitcast(F32),
                                      in_=rp[:, F:])
                if b1_p is not None:
                    # bias folded into the aggregated features (sum alpha = 1)
                    nc.vector.tensor_add(out=rs[:, :F], in0=rp[:, :F], in1=b1_sb[:])
                else:
                    nc.vector.tensor_copy(out=rs[:, :F], in_=rp[:, :F])
                nc.sync.dma_start(out=r1loc[nt * PW:(nt + 1) * PW, :A1], in_=rs[:])
                if nt == W - 1:
                    # pad rows: zero features, alpha_src=-200 (exp ~ 4e-18)
                    for lo in range(NPC, NPCP, PW):
                        nr = min(PW, NPCP - lo)
                        nc.sync.dma_start(out=r1loc[lo:lo + nr, :F],
                                          in_=pzA[:nr])
                        nc.sync.dma_start(
                            out=r1loc[lo:lo + nr, F:F + 64].bitcast(F32),
                            in_=pcA[:nr])
                    # single AllGather: per-op fixed cost (~25us) makes
                    # quarter-split collectives a net loss for this table.
                    nc.gpsimd.collective_compute(
                        "AllGather", ALU.bypass,
                        replica_groups=[list(range(CORES))],
                        ins=[r1loc[:, :].opt()],
                        outs=[tab1[:, :].opt()])

            # ---------------- phase B: layer-1 edge phase -----------------
            cc2_fired = [False] * NQ

            def fire_cc2(done_min):
                for q in range(NQ - 1, -1, -1):
                    if cc2_fired[q] or done_min > q * WQ:
                        continue
                    cc2_fired[q] = True
                    nc.gpsimd.collective_compute(
                        "AllGather", ALU.bypass,
                        replica_groups=[list(range(CORES))],
                        ins=[r2loc[q * QH:(q + 1) * QH, :].opt()],
                        outs=[tab2[q * CORES * QH:(q + 1) * CORES * QH, :].opt()])

            # self-loop attention terms for all windows in one shot:
            # exp(leaky_relu(a_s + a_d)) on [128, W, H]
            aall = cpool.tile([128, W, 16], F32)
            nc.sync.dma_start(
                out=aall[:],
                in_=r1loc[:W * PW, F:F + 64].bitcast(F32)
                    .rearrange("(w p) a -> p w a", p=PW))
            essall = cpool.tile([128, W, H], F32)
            nc.vector.tensor_tensor(out=essall[:], in0=aall[:, :, :H],
                                    in1=aall[:, :, H:], op=ALU.add)
            nc.vector.scalar_tensor_tensor(
                out=essall[:], in0=essall[:], scalar=NEG_SLOPE, in1=essall[:],
                op0=ALU.mult, op1=ALU.max)
            nc.scalar.activation(out=essall[:], in_=essall[:], func=ACTF.Exp)

            for w in order:
                kw = K[w]
                # the window's gather is split over the 4 SWDGE queues
                # (disjoint column ranges of one tile): descriptor generation
                # runs on all 4 Q7 core pairs concurrently (~3x), and the
                # per-window ring population keeps transfers window-serial so
                # consumers pipeline behind them.
                G1 = gp.tile([128, kw, REC1], FP8, tag="G1")
                qof = [kw * q // GRP for q in range(GRP + 1)]
                for q in range(GRP):
                    kq = qof[q + 1] - qof[q]
                    if kq == 0:
                        continue
                    nc.gpsimd.dma_gather(
                        G1[:, qof[q]:qof[q + 1], :], tab1[:, :],
                        isrc_sb[:, (CK[w] + qof[q]) * 8:(CK[w] + qof[q + 1]) * 8],
                        kq * PW, kq * PW, REC1, single_packet=False,
                        queue_num=q)
                if True:
                    loc = mp.tile([128, F], FP8, tag="loc")
                    nc.sync.dma_start(out=loc[:],
                                      in_=r1loc[w * PW:(w + 1) * PW, :F])
                    # ex = exp(leaky_relu(a_s[src] + a_d[dst]))  (pads -> ~0)
                    es_t = mp.tile([128, kw, H], F32, tag="es")
                    es = es_t[:, :, :]
                    nc.vector.tensor_tensor(
                        out=es, in0=G1[:, :, F:F + 32].bitcast(F32),
                        in1=aall[:, w, H:].unsqueeze(1)
                            .to_broadcast([128, kw, H]),
                        op=ALU.add)
                    nc.vector.scalar_tensor_tensor(
                        out=es, in0=es, scalar=NEG_SLOPE, in1=es,
                        op0=ALU.mult, op1=ALU.max)
                    nc.scalar.activation(out=es, in_=es, func=ACTF.Exp)
                    # alpha = ex / (sum_t ex + ex_self), folded to bf16 so the
                    # big multiply runs at the DVE 16-bit rate
                    den = mp.tile([128, H], F32, tag="den")
                    nc.vector.tensor_reduce(
                        out=den[:], in_=es.rearrange("p t h -> p h t"),
                        axis=AXX, op=ALU.add)
                    nc.vector.tensor_add(out=den[:], in0=den[:],
                                         in1=essall[:, w, :])
                    rcp = mp.tile([128, H], F32, tag="rcp")
                    nc.vector.reciprocal(rcp[:], den[:])
                    abf = mp.tile([128, kw, H], BF16, tag="abf")
                    nc.vector.tensor_tensor(
                        out=abf[:], in0=es,
                        in1=rcp[:].unsqueeze(1).to_broadcast([128, kw, H]),
                        op=ALU.mult)
                    asf = mp.tile([128, H], F32, tag="asf")
                    nc.vector.tensor_tensor(out=asf[:], in0=essall[:, w, :],
                                            in1=rcp[:], op=ALU.mult)
                    # weighted feature sum over edge columns + self
                    GW = mp.tile([128, kw, F], BF16, tag="GW")
                    nc.vector.tensor_tensor(
                        out=GW[:].rearrange("p t (h c) -> p t h c", h=H),
                        in0=G1[:, :, :F].rearrange("p t (h c) -> p t h c", h=H),
                        in1=abf[:].unsqueeze(3).to_broadcast([128, kw, H, C]),
                        op=ALU.mult)
                    # pairwise-tree segment sum over t: contiguous bf16 adds
                    # run at the DVE 16-bit rate (the strided f32 tensor_reduce
                    # over the t axis is ~2.5x slower)
                    n = kw
                    while n > 1:
                        if n % 2 == 1:
                            nc.vector.tensor_tensor(
                                out=GW[:, 0, :], in0=GW[:, 0, :],
                                in1=GW[:, n - 1, :], op=ALU.add)
                        h2 = n // 2
                        nc.vector.tensor_tensor(
                            out=GW[:, :h2, :], in0=GW[:, :h2, :],
                            in1=GW[:, h2:2 * h2, :], op=ALU.add)
                        n = h2
                    tmp = mp.tile([128, F], F32, tag="tmp")
                    nc.vector.tensor_tensor(
                        out=tmp[:].rearrange("p (h c) -> p h c", h=H),
                        in0=loc[:].rearrange("p (h c) -> p h c", h=H),
                        in1=asf[:].unsqueeze(2).to_broadcast([128, H, C]),
                        op=ALU.mult)
                    ho = mp.tile([128, F], F32, tag="ho")
                    nc.vector.tensor_add(out=ho[:], in0=GW[:, 0, :], in1=tmp[:])
                    # ELU(x) = relu(x) + exp(-relu(-x)) - 1, with all three
                    # activations on the (otherwise idle) scalar engine
                    xm = mp.tile([128, F], F32, tag="xm")
                    nc.scalar.activation(out=xm[:], in_=ho[:], func=ACTF.Relu,
                                         scale=-1.0)
                    nc.scalar.activation(out=xm[:], in_=xm[:], func=ACTF.Exp,
                                         scale=-1.0)
                    nc.scalar.activation(out=ho[:], in_=ho[:], func=ACTF.Relu)
                    nc.vector.scalar_tensor_tensor(
                        out=ho[:], in0=ho[:], scalar=-1.0, in1=xm[:],
                        op0=ALU.add, op1=ALU.add)
                    hT = mp.tile([128, KT, 128], F32, tag="hT")
                    for g in range(KT):
                        tp = psA.tile([128, 128], F32, tag="tp")
                        nc.tensor.transpose(out=tp[:],
                                            in_=ho[:, g * 128:(g + 1) * 128],
                                            identity=ident[:])
                        nc.vector.tensor_copy(out=hT[:, g, :], in_=tp[:])
                    r2p = psB.tile([128, A2M], F32, tag="acc2")
                    for g in range(KT):
                        nc.tensor.matmul(r2p[:], lhsT=hT[:, g, :],
                                         rhs=wc2_sb[:, g, :],
                                         start=(g == 0), stop=(g == KT - 1))
                    r2sb = mp.tile([128, A2], BF16, tag="r2sb")
                    nc.vector.tensor_copy(out=r2sb[:, :OUT], in_=r2p[:, :OUT])
                    nc.vector.tensor_copy(
                        out=r2sb[:, OUT:OUT + 4].bitcast(F32),
                        in_=r2p[:, OUT:OUT + 2])
                    nc.sync.dma_start(out=r2loc[w * PW:(w + 1) * PW, :A2],
                                      in_=r2sb[:])
                    if w == W - 1 and NPCP > NPC:
                        # pad rows (inside the last quarter's row range)
                        for lo in range(NPC, NPCP, PW):
                            nr = min(PW, NPCP - lo)
                            nc.sync.dma_start(out=r2loc[lo:lo + nr, :OUT],
                                              in_=pzB[:nr])
                            nc.sync.dma_start(
                                out=r2loc[lo:lo + nr, OUT:OUT + 4].bitcast(F32),
                                in_=pcB[:nr])
                fire_cc2(w)

            # ---------------- phase D: layer-2 edge phase -----------------
            # batched self-loop attention terms from the rec2 alpha columns
            a2all = cpool.tile([128, W, 2], F32)
            nc.sync.dma_start(
                out=a2all[:],
                in_=r2loc[:W * PW, OUT:OUT + 4].bitcast(F32)
                    .rearrange("(w p) a -> p w a", p=PW))
            ess2all = cpool.tile([128, W], F32)
            nc.vector.tensor_tensor(out=ess2all[:], in0=a2all[:, :, 0],
                                    in1=a2all[:, :, 1], op=ALU.add)
            nc.vector.scalar_tensor_tensor(
                out=ess2all[:], in0=ess2all[:], scalar=NEG_SLOPE,
                in1=ess2all[:], op0=ALU.mult, op1=ALU.max)
            nc.scalar.activation(out=ess2all[:], in_=ess2all[:], func=ACTF.Exp)

            for w in order:
                kw = K[w]
                G2 = gp.tile([128, kw, REC2], BF16, tag="G2")
                qof = [kw * q // GRP for q in range(GRP + 1)]
                for q in range(GRP):
                    kq = qof[q + 1] - qof[q]
                    if kq == 0:
                        continue
                    nc.gpsimd.dma_gather(
                        G2[:, qof[q]:qof[q + 1], :], tab2[:, :],
                        isrc2_sb[:, (CK[w] + qof[q]) * 8:(CK[w] + qof[q + 1]) * 8],
                        kq * PW, kq * PW, REC2, single_packet=False,
                        queue_num=q)
                if True:
                    loc2 = mp.tile([128, OUT], BF16, tag="loc2")
                    nc.sync.dma_start(out=loc2[:],
                                      in_=r2loc[w * PW:(w + 1) * PW, :OUT])
                    es2_t = mp.tile([128, kw, 1], F32, tag="es2")
                    es2 = es2_t[:, :, :]
                    nc.vector.tensor_tensor(
                        out=es2, in0=G2[:, :, OUT:OUT + 2].bitcast(F32),
                        in1=a2all[:, w, 1:2].unsqueeze(1)
                            .to_broadcast([128, kw, 1]),
                        op=ALU.add)
                    nc.vector.scalar_tensor_tensor(
                        out=es2, in0=es2, scalar=NEG_SLOPE, in1=es2,
                        op0=ALU.mult, op1=ALU.max)
                    nc.scalar.activation(out=es2, in_=es2, func=ACTF.Exp)
                    den2 = mp.tile([128, 1], F32, tag="den2")
                    nc.vector.tensor_reduce(
                        out=den2[:], in_=es2.rearrange("p t h -> p h t"),
                        axis=AXX, op=ALU.add)
                    nc.vector.tensor_add(out=den2[:], in0=den2[:],
                                         in1=ess2all[:, w:w + 1])
                    rcp2 = mp.tile([128, 1], F32, tag="rcp2")
                    nc.vector.reciprocal(rcp2[:], den2[:])
                    nc.vector.tensor_scalar_mul(out=es2, in0=es2,
                                                scalar1=rcp2[:, :1])
                    nc.vector.tensor_tensor(
                        out=G2[:, :, :OUT], in0=G2[:, :, :OUT],
                        in1=es2.to_broadcast([128, kw, OUT]), op=ALU.mult)
                    num2 = mp.tile([128, OUT], F32, tag="num2")
                    nc.vector.tensor_reduce(
                        out=num2[:], in_=G2[:, :, :OUT].rearrange("p t f -> p f t"),
                        axis=AXX, op=ALU.add)
                    tmp2 = mp.tile([128, OUT], F32, tag="tmp2")
                    nc.vector.tensor_scalar(
                        out=tmp2[:], in0=loc2[:], scalar1=ess2all[:, w:w + 1],
                        scalar2=rcp2[:, :1], op0=ALU.mult, op1=ALU.mult)
                    nc.vector.tensor_add(out=lgall[:, w, :], in0=num2[:],
                                         in1=tmp2[:])
                    if b2_p is not None:
                        nc.vector.tensor_add(out=lgall[:, w, :],
                                             in0=lgall[:, w, :], in1=b2_sb[:])

            # batched log_softmax over all windows (no max-shift: logits are
            # bounded ~[-2, 2]); one ACT table load for the W Exps, one Ln.
            for w in range(W):
                nc.scalar.activation(out=ezsc[:], in_=lgall[:, w, :],
                                     func=ACTF.Exp, accum_out=ssall[:, w:w + 1])
            nc.scalar.activation(out=lsall[:], in_=ssall[:], func=ACTF.Ln)
            nc.vector.tensor_tensor(
                out=lgall[:], in0=lgall[:],
                in1=lsall[:].unsqueeze(2).to_broadcast([128, W, OUT]),
                op=ALU.subtract)
            nc.sync.dma_start(
                out=out_p[:W * PW].rearrange("(w p) o -> p w o", p=PW),
                in_=lgall[:])

    nc.compile()
    return nc


# --------------------------------------------------------------------------
# public entry point
# --------------------------------------------------------------------------

def kernel(x, edge_index, W1, a1_src, a1_dst, b1, W2, a2_src, a2_dst, b2,
           _want_trace=False):
    x = np.asarray(x)
    host, in_maps = _prep(x, np.asarray(edge_index), np.asarray(W1),
                          np.asarray(a1_src), np.asarray(a1_dst),
                          np.asarray(b1), np.asarray(W2), np.asarray(a2_src),
                          np.asarray(a2_dst), np.asarray(b2))
    key = (host["N"], host["F"], host["H"], host["C"], host["OUT"],
           host["K"], host["NQ"], host["use_b1"], host["use_b2"])
    if key not in _CACHE:
        _CACHE[key] = _build(host)
    nc = _CACHE[key]
    res = run_bass_kernel_spmd(nc, in_maps, core_ids=list(range(CORES)),
                               trace=_want_trace)
    NPC = host["NPC"]
    out = np.empty((host["N"], host["OUT"]), np.float32)
    for c in range(CORES):
        out[host["perm"][c * NPC:(c + 1) * NPC]] = res.results[c]["out"][:NPC]
    if _want_trace:
        kernel._last_result = res
    return np.ascontiguousarray(out)
